# revision 14
# baseline (speedup 1.0000x reference)
"""Multi-head attention (B=4, Q=K=2048, N=12 heads, H=64) on 8 TRN2 NeuronCores.

Sharding: core c handles batch b = c // 2 and head-group g = c % 2 (6 heads,
output columns [g*384:(g+1)*384]).  Pure data-parallel: no collectives.

Per-core kernel ("transposed flash attention"):
  - x_q / x_k / x_v row-shards are PE-transposed into xT slabs [d, rows].
  - qT[h, q] and kT[h, k] come out of the projections directly (lhsT = W tile,
    rhs = xT tile), so the score matmul needs no other transposes:
        sT[k_tile, q_block] = kT_h[:, k_tile].T @ qT_h[:, q_block]
  - exp is fused into the PSUM->SBUF eviction on the scalar engine
    (out = Exp(0.125 * s)); no row-max subtraction (scores are O(1) for this
    input distribution, so exp is safe in fp32).
  - v is projected in natural [k, h] layout with a ones column appended; the
    PV matmul then yields both the unnormalized context and the softmax
    denominator in a single pass:
        cT_aug[0:64, q] = sum_k v[k, h] * e[k, q],  cT_aug[64, q] = sum_k e[k, q]
  - cT_aug [65, q] blocks are PE-transposed back to [q, 65], divided by the
    denominator (per-partition scalar), assembled to [128, 384] row tiles and
    DMA'd out.
  - matmuls run as float32r (full PE rate for free dim >= 256).
"""

import sys
from contextlib import ExitStack

sys.path.insert(0, "/opt/trn_rl_repo")

import numpy as np

import concourse.bass as bass
import concourse.tile as tile
from concourse import bacc, mybir
from concourse.bass_utils import run_bass_kernel_spmd

F32 = mybir.dt.float32
F32R = mybir.dt.float32r

B, SEQ, N_HEADS, H = 4, 2048, 12, 64
D = N_HEADS * H            # 768
NH = 6                     # heads per core
DG = NH * H                # 384 output cols per core
P = 128
DT = D // P                # 6 d-tiles
RT = SEQ // P              # 16 row tiles (q and k)
QB = SEQ // 512            # 4 q blocks of 512
CHUNKS = [3, 3, 3, 3, 3, 1]  # k-tile chunking of the exp/PV pipeline
SCALE = 0.125              # 1/sqrt(64)


def r32(ap):
    return ap.bitcast(F32R)


def build_nc(has_bias: bool, has_mask: bool, reps: int = 1):
    nc = bacc.Bacc("TRN2", target_bir_lowering=False, debug=False, num_devices=8)

    xq = nc.dram_tensor("xq", [SEQ, D], F32R, kind="ExternalInput").ap()
    xk = nc.dram_tensor("xk", [SEQ, D], F32R, kind="ExternalInput").ap()
    xv = nc.dram_tensor("xv", [SEQ, D], F32R, kind="ExternalInput").ap()
    wq = nc.dram_tensor("wq", [D, DG], F32R, kind="ExternalInput").ap()
    wk = nc.dram_tensor("wk", [D, DG], F32R, kind="ExternalInput").ap()
    wv = nc.dram_tensor("wv", [D, DG], F32R, kind="ExternalInput").ap()
    if has_bias:
        bq = nc.dram_tensor("bq", [DG], F32, kind="ExternalInput").ap()
        bk = nc.dram_tensor("bk", [DG], F32, kind="ExternalInput").ap()
        bv = nc.dram_tensor("bv", [DG], F32, kind="ExternalInput").ap()
    if has_mask:
        # mask^T * 8, so exp(0.125*(s + m8)) == exp(s/8 + mask)
        m8t = nc.dram_tensor("m8t", [SEQ, SEQ], F32, kind="ExternalInput").ap()
    ident_d = nc.dram_tensor("ident", [P, P], F32R, kind="ExternalInput").ap()
    ones_d = nc.dram_tensor("ones", [P, NH], F32R, kind="ExternalInput").ap()
    out = nc.dram_tensor("out", [SEQ, DG], F32, kind="ExternalOutput").ap()

    with tile.TileContext(nc) as tc:
      for _rep in range(reps):
       with ExitStack() as stack:
        singles = stack.enter_context(tc.tile_pool(name="singles", bufs=1))
        ident = singles.tile([P, P], F32R)
        nc.sync.dma_start(out=ident, in_=ident_d)

        # weight tiles [128, 384] per d-tile
        w_sb = {}
        for name, w in (("q", wq), ("k", wk), ("v", wv)):
            for dt in range(DT):
                t = singles.tile([P, DG], F32R, tag=f"w{name}{dt}")
                nc.sync.dma_start(out=t, in_=w[dt * P:(dt + 1) * P, :])
                w_sb[name, dt] = t
        b_sb = {}
        if has_bias:
            for name, b in (("q", bq), ("k", bk)):
                for m in range(DG // P):
                    t = singles.tile([P, 1], F32, tag=f"b{name}{m}")
                    nc.sync.dma_start(out=t, in_=b[m * P:(m + 1) * P][:, None])
                    b_sb[name, m] = t
            bv_bc = singles.tile([P, DG], F32)
            nc.sync.dma_start(out=bv_bc, in_=bv.to_broadcast((P, DG)))

        xrow = stack.enter_context(tc.tile_pool(name="xrow", bufs=4))
        xT = stack.enter_context(tc.tile_pool(name="xT", bufs=1))
        qkT = stack.enter_context(tc.tile_pool(name="qkT", bufs=1))
        vpool = stack.enter_context(tc.tile_pool(name="v", bufs=1))

        def load_transpose(x_ap, which, psum_pool, ppsum_tag):
            """DMA x [2048, 768] and produce xT slabs [128, 2048] per d-tile."""
            slabs = [xT.tile([P, SEQ], F32R, tag=f"xT{dt}",
                              name=f"xT{which}{dt}") for dt in range(DT)]
            for rt in range(RT):
                xr = xrow.tile([P, D], F32R, tag="xrow")
                nc.sync.dma_start(out=xr, in_=x_ap[rt * P:(rt + 1) * P, :])
                for dt in range(DT):
                    tag = ppsum_tag if ppsum_tag else f"s{dt % 2}"
                    tp = psum_pool.tile([P, P], F32R, tag=tag, name="tp")
                    nc.tensor.transpose(tp, xr[:, dt * P:(dt + 1) * P], ident)
                    nc.vector.tensor_copy(out=slabs[dt][:, rt * P:(rt + 1) * P],
                                          in_=tp)
            return slabs

        def project_qk(slabs, which, psum_pool):
            """qT / kT slabs [128, 2048]: 3 m-tiles of 2 heads each."""
            outs = []
            for m in range(DG // P):
                dst = qkT.tile([P, SEQ], F32R, tag=f"{which}T{m}")
                for qb in range(QB):
                    pj = psum_pool.tile([P, 512], F32, tag="pj")
                    for dt in range(DT):
                        nc.tensor.matmul(
                            pj, w_sb[which, dt][:, m * P:(m + 1) * P],
                            slabs[dt][:, qb * 512:(qb + 1) * 512],
                            start=(dt == 0), stop=(dt == DT - 1))
                    dslice = dst[:, qb * 512:(qb + 1) * 512]
                    if has_bias:
                        nc.vector.tensor_scalar_add(dslice, pj, b_sb[which, m])
                    else:
                        nc.vector.tensor_copy(out=dslice, in_=pj)
                outs.append(dst)
            return outs

        # ---- phase A: q/k transposes + projections --------------------------
        with tc.tile_pool(name="psA", bufs=2, space="PSUM") as psA:
            k_slabs = load_transpose(xk, "k", psA, "tp")
            q_slabs = load_transpose(xq, "q", psA, "tp")
            kT = project_qk(k_slabs, "k", psA)
            qT = project_qk(q_slabs, "q", psA)

        # ---- phase B pools --------------------------------------------------
        psS = stack.enter_context(tc.tile_pool(name="psS", bufs=1, space="PSUM"))
        psPV = stack.enter_context(tc.tile_pool(name="psPV", bufs=1, space="PSUM"))
        expp = stack.enter_context(tc.tile_pool(name="expp", bufs=2))
        cTp = stack.enter_context(tc.tile_pool(name="cT", bufs=2))
        outp = stack.enter_context(tc.tile_pool(name="outp", bufs=2))
        smallp = stack.enter_context(tc.tile_pool(name="small", bufs=4))

        # v transposes + projection (shares the psPV bank slots)
        v_slabs = load_transpose(xv, "v", psS, None)
        v_sb = []
        for kt in range(RT):
            vt = vpool.tile([P, NH, H + 1], F32R, tag=f"v{kt}")
            pj = psPV.tile([P, DG], F32, tag=f"pv{kt % 2}", name="vproj")
            for dt in range(DT):
                nc.tensor.matmul(pj, v_slabs[dt][:, kt * P:(kt + 1) * P],
                                 w_sb["v", dt],
                                 start=(dt == 0), stop=(dt == DT - 1))
            dst = vt[:, :, 0:H]
            pjv = pj.rearrange("p (n h) -> p n h", h=H)
            if has_bias:
                nc.vector.tensor_add(dst, pjv, bv_bc.rearrange("p (n h) -> p n h", h=H))
            else:
                nc.vector.tensor_copy(out=dst, in_=pjv)
            nc.sync.dma_start(out=vt[:, :, H], in_=ones_d)
            v_sb.append(vt)

        # ---- phase B: flash attention main loop -----------------------------
        if has_mask:
            maskp = stack.enter_context(tc.tile_pool(name="maskp", bufs=4))

        HPS = (0, 1)
        for qb in range(QB):
            out_tiles = [outp.tile([P, DG], F32, tag=f"out{st}", name=f"out{st}")
                         for st in range(4)]
            for m in range(NH // 2):    # head pairs: the projection m-tiles
                kTh = {hp: kT[m][hp * H:(hp + 1) * H, :] for hp in HPS}
                qTh = {hp: qT[m][hp * H:(hp + 1) * H, qb * 512:(qb + 1) * 512]
                       for hp in HPS}
                pv = {hp: psPV.tile([H + 1, 512], F32, tag=f"pv{hp}",
                                    name=f"pv{hp}") for hp in HPS}
                kt0 = 0
                for csz in CHUNKS:
                    s = {hp: psS.tile([P, 1536], F32, tag=f"s{hp}",
                                      name=f"s{hp}") for hp in HPS}
                    # two heads run concurrently in distinct PE row groups
                    for j in range(csz):
                        kt = kt0 + j
                        for hp in HPS:
                            nc.tensor.matmul(
                                s[hp][:, j * 512:(j + 1) * 512],
                                kTh[hp][:, kt * P:(kt + 1) * P], qTh[hp],
                                start=True, stop=True,
                                tile_position=(hp * H, 0))
                    if has_mask:
                        mt = maskp.tile([P, csz, 512], F32, tag="mask")
                        nc.sync.dma_start(
                            out=mt,
                            in_=m8t[kt0 * P:(kt0 + csz) * P,
                                    qb * 512:(qb + 1) * 512].rearrange(
                                        "(c p) q -> p c q", p=P))
                        for hp in HPS:
                            sv = s[hp][:, 0:csz * 512].rearrange(
                                "p (c q) -> p c q", q=512)
                            nc.vector.tensor_add(sv, sv, mt)
                    e = {}
                    for hp in HPS:
                        e[hp] = expp.tile([P, 1536], F32R, tag=f"exp{hp}",
                                          name=f"exp{hp}")
                        nc.scalar.activation(
                            out=e[hp][:, 0:csz * 512], in_=s[hp][:, 0:csz * 512],
                            func=mybir.ActivationFunctionType.Exp, scale=SCALE)
                    for hp in HPS:
                        for j in range(csz):
                            kt = kt0 + j
                            nc.tensor.matmul(
                                pv[hp], v_sb[kt][:, 2 * m + hp, :],
                                e[hp][:, j * 512:(j + 1) * 512],
                                start=(kt == 0), stop=(kt == RT - 1))
                    kt0 += csz
                for hp in HPS:
                    head = 2 * m + hp
                    cT = cTp.tile([P, 512], F32R, tag=f"cT{hp}", name=f"cT{hp}")
                    nc.vector.tensor_copy(out=cT[0:H + 1, :], in_=pv[hp])
                    for st in range(4):
                        ctp = psPV.tile([P, P], F32R, tag=f"pv{hp}",
                                        name=f"ctp{hp}")
                        nc.tensor.transpose(ctp, cT[:, st * P:(st + 1) * P], ident)
                        rec = smallp.tile([P, 1], F32, tag="rec")
                        nc.vector.reciprocal(rec, ctp[:, H:H + 1].bitcast(F32))
                        nc.vector.tensor_scalar_mul(
                            out_tiles[st][:, head * H:(head + 1) * H],
                            ctp[:, 0:H].bitcast(F32), rec)
            for st in range(4):
                nc.sync.dma_start(
                    out=out[qb * 512 + st * P: qb * 512 + (st + 1) * P, :],
                    in_=out_tiles[st])

    nc.compile()
    return nc


_NC_CACHE = {}


def _get_nc(has_bias, has_mask, reps=1):
    key = (has_bias, has_mask, reps)
    if key not in _NC_CACHE:
        _NC_CACHE[key] = build_nc(has_bias, has_mask, reps)
    return _NC_CACHE[key]


def shard_inputs(query, key, value, mask, Wq, bq, Wk, bk, Wv, bv,
                 batch_size=B, num_heads=N_HEADS):
    query = np.ascontiguousarray(np.asarray(query, dtype=np.float32))
    key = np.ascontiguousarray(np.asarray(key, dtype=np.float32))
    value = np.ascontiguousarray(np.asarray(value, dtype=np.float32))
    Wq = np.asarray(Wq, dtype=np.float32)
    Wk = np.asarray(Wk, dtype=np.float32)
    Wv = np.asarray(Wv, dtype=np.float32)
    bq = np.asarray(bq, dtype=np.float32)
    bk = np.asarray(bk, dtype=np.float32)
    bv = np.asarray(bv, dtype=np.float32)
    mask = np.asarray(mask, dtype=np.float32)
    assert query.shape == (B * SEQ, D) and key.shape == (B * SEQ, D)
    assert int(batch_size) == B and int(num_heads) == N_HEADS

    has_bias = bool(np.any(bq) or np.any(bk) or np.any(bv))
    has_mask = bool(np.any(mask))

    in_maps = []
    for c in range(8):
        b, g = divmod(c, 2)
        rows = slice(b * SEQ, (b + 1) * SEQ)
        cols = slice(g * DG, (g + 1) * DG)
        m = {
            "ident": np.eye(P, dtype=np.float32),
            "ones": np.ones((P, NH), dtype=np.float32),
            "xq": query[rows],
            "xk": key[rows],
            "xv": value[rows],
            "wq": np.ascontiguousarray(Wq[:, cols]),
            "wk": np.ascontiguousarray(Wk[:, cols]),
            "wv": np.ascontiguousarray(Wv[:, cols]),
        }
        if has_bias:
            m["bq"] = np.ascontiguousarray(bq[cols])
            m["bk"] = np.ascontiguousarray(bk[cols])
            m["bv"] = np.ascontiguousarray(bv[cols])
        if has_mask:
            m["m8t"] = np.ascontiguousarray(mask[b, 0].T * 8.0)
        in_maps.append(m)
    return in_maps, has_bias, has_mask


def make_in_maps(inputs):
    return shard_inputs(**{k: inputs[k] for k in
                           ("query", "key", "value", "mask", "Wq", "bq",
                            "Wk", "bk", "Wv", "bv", "batch_size", "num_heads")})[0]


def assemble(results):
    full = np.empty((B * SEQ, D), dtype=np.float32)
    for c in range(8):
        b, g = divmod(c, 2)
        full[b * SEQ:(b + 1) * SEQ, g * DG:(g + 1) * DG] = results[c]["out"]
    return full


def kernel(query, key, value, mask, Wq, bq, Wk, bk, Wv, bv,
           batch_size=B, num_heads=N_HEADS, _trace=False, _trace_kwargs=None):
    in_maps, has_bias, has_mask = shard_inputs(
        query, key, value, mask, Wq, bq, Wk, bk, Wv, bv, batch_size, num_heads)
    nc = _get_nc(has_bias, has_mask)
    res = run_bass_kernel_spmd(nc, in_maps, list(range(8)), trace=_trace,
                               **(_trace_kwargs or {}))
    full = assemble(res.results)
    if _trace:
        return full, res
    return full


# revision 15
# speedup vs baseline: 2.4697x; 2.4697x over previous
"""Multi-head attention (B=4, Q=K=2048, N=12 heads, H=64) on 8 TRN2 NeuronCores.

Sharding: core c handles batch b = c // 2 and head-group g = c % 2 (6 heads,
output columns [g*384:(g+1)*384]).  Pure data-parallel: no collectives.

Per-core kernel ("transposed flash attention"):
  - x_q / x_k / x_v row-shards are PE-transposed into xT slabs [d, rows].
  - qT[h, q] and kT[h, k] come out of the projections directly (lhsT = W tile,
    rhs = xT tile), so the score matmul needs no other transposes:
        sT[k_tile, q_block] = kT_h[:, k_tile].T @ qT_h[:, q_block]
  - exp is fused into the PSUM->SBUF eviction on the scalar engine
    (out = Exp(0.125 * s)); no row-max subtraction (scores are O(1) for this
    input distribution, so exp is safe in fp32).
  - v is projected in natural [k, h] layout with a ones column appended; the
    PV matmul then yields both the unnormalized context and the softmax
    denominator in a single pass:
        cT_aug[0:64, q] = sum_k v[k, h] * e[k, q],  cT_aug[64, q] = sum_k e[k, q]
  - cT_aug [65, q] blocks are PE-transposed back to [q, 65], divided by the
    denominator (per-partition scalar), assembled to [128, 384] row tiles and
    DMA'd out.
  - matmuls run as float32r (full PE rate for free dim >= 256).
"""

import sys
from contextlib import ExitStack

sys.path.insert(0, "/opt/trn_rl_repo")

import numpy as np

import concourse.bass as bass
import concourse.tile as tile
from concourse import bacc, mybir
from concourse.bass_utils import run_bass_kernel_spmd

F32 = mybir.dt.float32
F32R = mybir.dt.float32r

B, SEQ, N_HEADS, H = 4, 2048, 12, 64
D = N_HEADS * H            # 768
NH = 6                     # heads per core
DG = NH * H                # 384 output cols per core
P = 128
DT = D // P                # 6 d-tiles
RT = SEQ // P              # 16 row tiles (q and k)
QB = SEQ // 512            # 4 q blocks of 512
CHUNKS = [3, 3, 3, 3, 3, 1]  # k-tile chunking of the exp/PV pipeline
SCALE = 0.125              # 1/sqrt(64)


def r32(ap):
    return ap.bitcast(F32R)


def build_nc(has_bias: bool, has_mask: bool, reps: int = 1):
    nc = bacc.Bacc("TRN2", target_bir_lowering=False, debug=False, num_devices=8)

    xq = nc.dram_tensor("xq", [SEQ, D], F32R, kind="ExternalInput").ap()
    xk = nc.dram_tensor("xk", [SEQ, D], F32R, kind="ExternalInput").ap()
    xv = nc.dram_tensor("xv", [SEQ, D], F32R, kind="ExternalInput").ap()
    wq = nc.dram_tensor("wq", [D, DG], F32R, kind="ExternalInput").ap()
    wk = nc.dram_tensor("wk", [D, DG], F32R, kind="ExternalInput").ap()
    wv = nc.dram_tensor("wv", [D, DG], F32R, kind="ExternalInput").ap()
    if has_bias:
        bq = nc.dram_tensor("bq", [DG], F32, kind="ExternalInput").ap()
        bk = nc.dram_tensor("bk", [DG], F32, kind="ExternalInput").ap()
        bv = nc.dram_tensor("bv", [DG], F32, kind="ExternalInput").ap()
    if has_mask:
        # mask^T * 8, so exp(0.125*(s + m8)) == exp(s/8 + mask)
        m8t = nc.dram_tensor("m8t", [SEQ, SEQ], F32, kind="ExternalInput").ap()
    ident_d = nc.dram_tensor("ident", [P, P], F32R, kind="ExternalInput").ap()
    ones_d = nc.dram_tensor("ones", [P, NH], F32R, kind="ExternalInput").ap()
    out = nc.dram_tensor("out", [SEQ, DG], F32, kind="ExternalOutput").ap()

    with tile.TileContext(nc) as tc:
      for _rep in range(reps):
       with ExitStack() as stack:
        singles = stack.enter_context(tc.tile_pool(name="singles", bufs=1))
        ident = singles.tile([P, P], F32R)
        nc.sync.dma_start(out=ident, in_=ident_d)

        # weight tiles [128, 384] per d-tile
        w_sb = {}
        for name, w in (("q", wq), ("k", wk), ("v", wv)):
            for dt in range(DT):
                t = singles.tile([P, DG], F32R, tag=f"w{name}{dt}")
                nc.sync.dma_start(out=t, in_=w[dt * P:(dt + 1) * P, :])
                w_sb[name, dt] = t
        b_sb = {}
        if has_bias:
            for name, b in (("q", bq), ("k", bk)):
                for m in range(DG // P):
                    t = singles.tile([P, 1], F32, tag=f"b{name}{m}")
                    nc.sync.dma_start(out=t, in_=b[m * P:(m + 1) * P][:, None])
                    b_sb[name, m] = t
            bv_bc = singles.tile([P, DG], F32)
            nc.sync.dma_start(out=bv_bc, in_=bv.to_broadcast((P, DG)))

        xrow = stack.enter_context(tc.tile_pool(name="xrow", bufs=4))
        xT = stack.enter_context(tc.tile_pool(name="xT", bufs=1))
        qkT = stack.enter_context(tc.tile_pool(name="qkT", bufs=1))
        vpool = stack.enter_context(tc.tile_pool(name="v", bufs=1))

        def load_transpose(x_ap, which, psum_pool, ppsum_tag):
            """DMA x [2048, 768] and produce xT slabs [128, 2048] per d-tile."""
            slabs = [xT.tile([P, SEQ], F32R, tag=f"xT{dt}",
                              name=f"xT{which}{dt}") for dt in range(DT)]
            for rt in range(RT):
                xr = xrow.tile([P, D], F32R, tag="xrow")
                nc.sync.dma_start(out=xr, in_=x_ap[rt * P:(rt + 1) * P, :])
                for dt in range(DT):
                    tag = ppsum_tag if ppsum_tag else f"s{dt % 2}"
                    tp = psum_pool.tile([P, P], F32R, tag=tag, name="tp")
                    nc.tensor.transpose(tp, xr[:, dt * P:(dt + 1) * P], ident)
                    nc.vector.tensor_copy(out=slabs[dt][:, rt * P:(rt + 1) * P],
                                          in_=tp)
            return slabs

        def project_qk(slabs, which, psum_pool):
            """qT / kT slabs [128, 2048]: 3 m-tiles of 2 heads each."""
            outs = []
            for m in range(DG // P):
                dst = qkT.tile([P, SEQ], F32R, tag=f"{which}T{m}")
                for qb in range(QB):
                    pj = psum_pool.tile([P, 512], F32, tag="pj")
                    for dt in range(DT):
                        nc.tensor.matmul(
                            pj, w_sb[which, dt][:, m * P:(m + 1) * P],
                            slabs[dt][:, qb * 512:(qb + 1) * 512],
                            start=(dt == 0), stop=(dt == DT - 1))
                    dslice = dst[:, qb * 512:(qb + 1) * 512]
                    if has_bias:
                        nc.vector.tensor_scalar_add(dslice, pj, b_sb[which, m])
                    else:
                        nc.vector.tensor_copy(out=dslice, in_=pj)
                outs.append(dst)
            return outs

        # ---- phase A: q/k transposes + projections --------------------------
        with tc.tile_pool(name="psA", bufs=2, space="PSUM") as psA:
            k_slabs = load_transpose(xk, "k", psA, "tp")
            q_slabs = load_transpose(xq, "q", psA, "tp")
            kT = project_qk(k_slabs, "k", psA)
            qT = project_qk(q_slabs, "q", psA)

        # ---- phase B pools --------------------------------------------------
        psS = stack.enter_context(tc.tile_pool(name="psS", bufs=1, space="PSUM"))
        psPV = stack.enter_context(tc.tile_pool(name="psPV", bufs=1, space="PSUM"))
        expp = stack.enter_context(tc.tile_pool(name="expp", bufs=2))
        cTp = stack.enter_context(tc.tile_pool(name="cT", bufs=2))
        outp = stack.enter_context(tc.tile_pool(name="outp", bufs=2))
        smallp = stack.enter_context(tc.tile_pool(name="small", bufs=4))

        # v transposes + projection (shares the psPV bank slots)
        v_slabs = load_transpose(xv, "v", psS, None)
        v_sb = []
        for kt in range(RT):
            vt = vpool.tile([P, NH, H + 1], F32R, tag=f"v{kt}")
            pj = psPV.tile([P, DG], F32, tag=f"pv{kt % 2}", name="vproj")
            for dt in range(DT):
                nc.tensor.matmul(pj, v_slabs[dt][:, kt * P:(kt + 1) * P],
                                 w_sb["v", dt],
                                 start=(dt == 0), stop=(dt == DT - 1))
            dst = vt[:, :, 0:H]
            pjv = pj.rearrange("p (n h) -> p n h", h=H)
            if has_bias:
                nc.vector.tensor_add(dst, pjv, bv_bc.rearrange("p (n h) -> p n h", h=H))
            else:
                nc.vector.tensor_copy(out=dst, in_=pjv)
            nc.sync.dma_start(out=vt[:, :, H], in_=ones_d)
            v_sb.append(vt)

        # ---- phase B: flash attention main loop -----------------------------
        if has_mask:
            maskp = stack.enter_context(tc.tile_pool(name="maskp", bufs=4))

        HPS = (0, 1)
        for qb in range(QB):
            out_tiles = [outp.tile([P, DG], F32, tag=f"out{st}", name=f"out{st}")
                         for st in range(4)]
            for m in range(NH // 2):    # head pairs: the projection m-tiles
                kTh = {hp: kT[m][hp * H:(hp + 1) * H, :] for hp in HPS}
                qTh = {hp: qT[m][hp * H:(hp + 1) * H, qb * 512:(qb + 1) * 512]
                       for hp in HPS}
                pv = {hp: psPV.tile([H + 1, 512], F32, tag=f"pv{hp}",
                                    name=f"pv{hp}") for hp in HPS}
                kt0 = 0
                for csz in CHUNKS:
                    s = {hp: psS.tile([P, 1536], F32, tag=f"s{hp}",
                                      name=f"s{hp}") for hp in HPS}
                    # two heads run concurrently in distinct PE row groups
                    for j in range(csz):
                        kt = kt0 + j
                        for hp in HPS:
                            nc.tensor.matmul(
                                s[hp][:, j * 512:(j + 1) * 512],
                                kTh[hp][:, kt * P:(kt + 1) * P], qTh[hp],
                                start=True, stop=True)
                    if has_mask:
                        mt = maskp.tile([P, csz, 512], F32, tag="mask")
                        nc.sync.dma_start(
                            out=mt,
                            in_=m8t[kt0 * P:(kt0 + csz) * P,
                                    qb * 512:(qb + 1) * 512].rearrange(
                                        "(c p) q -> p c q", p=P))
                        for hp in HPS:
                            sv = s[hp][:, 0:csz * 512].rearrange(
                                "p (c q) -> p c q", q=512)
                            nc.vector.tensor_add(sv, sv, mt)
                    e = {}
                    for hp in HPS:
                        e[hp] = expp.tile([P, 1536], F32R, tag=f"exp{hp}",
                                          name=f"exp{hp}")
                        nc.scalar.activation(
                            out=e[hp][:, 0:csz * 512], in_=s[hp][:, 0:csz * 512],
                            func=mybir.ActivationFunctionType.Exp, scale=SCALE)
                    for hp in HPS:
                        for j in range(csz):
                            kt = kt0 + j
                            nc.tensor.matmul(
                                pv[hp], v_sb[kt][:, 2 * m + hp, :],
                                e[hp][:, j * 512:(j + 1) * 512],
                                start=(kt == 0), stop=(kt == RT - 1))
                    kt0 += csz
                for hp in HPS:
                    head = 2 * m + hp
                    cT = cTp.tile([P, 512], F32R, tag=f"cT{hp}", name=f"cT{hp}")
                    nc.vector.tensor_copy(out=cT[0:H + 1, :], in_=pv[hp])
                    for st in range(4):
                        ctp = psPV.tile([P, P], F32R, tag=f"pv{hp}",
                                        name=f"ctp{hp}")
                        nc.tensor.transpose(ctp, cT[:, st * P:(st + 1) * P], ident)
                        rec = smallp.tile([P, 1], F32, tag="rec")
                        nc.vector.reciprocal(rec, ctp[:, H:H + 1].bitcast(F32))
                        nc.vector.tensor_scalar_mul(
                            out_tiles[st][:, head * H:(head + 1) * H],
                            ctp[:, 0:H].bitcast(F32), rec)
            for st in range(4):
                nc.sync.dma_start(
                    out=out[qb * 512 + st * P: qb * 512 + (st + 1) * P, :],
                    in_=out_tiles[st])

    nc.compile()
    return nc


_NC_CACHE = {}


def _get_nc(has_bias, has_mask, reps=1):
    key = (has_bias, has_mask, reps)
    if key not in _NC_CACHE:
        _NC_CACHE[key] = build_nc(has_bias, has_mask, reps)
    return _NC_CACHE[key]


def shard_inputs(query, key, value, mask, Wq, bq, Wk, bk, Wv, bv,
                 batch_size=B, num_heads=N_HEADS):
    query = np.ascontiguousarray(np.asarray(query, dtype=np.float32))
    key = np.ascontiguousarray(np.asarray(key, dtype=np.float32))
    value = np.ascontiguousarray(np.asarray(value, dtype=np.float32))
    Wq = np.asarray(Wq, dtype=np.float32)
    Wk = np.asarray(Wk, dtype=np.float32)
    Wv = np.asarray(Wv, dtype=np.float32)
    bq = np.asarray(bq, dtype=np.float32)
    bk = np.asarray(bk, dtype=np.float32)
    bv = np.asarray(bv, dtype=np.float32)
    mask = np.asarray(mask, dtype=np.float32)
    assert query.shape == (B * SEQ, D) and key.shape == (B * SEQ, D)
    assert int(batch_size) == B and int(num_heads) == N_HEADS

    has_bias = bool(np.any(bq) or np.any(bk) or np.any(bv))
    has_mask = bool(np.any(mask))

    in_maps = []
    for c in range(8):
        b, g = divmod(c, 2)
        rows = slice(b * SEQ, (b + 1) * SEQ)
        cols = slice(g * DG, (g + 1) * DG)
        m = {
            "ident": np.eye(P, dtype=np.float32),
            "ones": np.ones((P, NH), dtype=np.float32),
            "xq": query[rows],
            "xk": key[rows],
            "xv": value[rows],
            "wq": np.ascontiguousarray(Wq[:, cols]),
            "wk": np.ascontiguousarray(Wk[:, cols]),
            "wv": np.ascontiguousarray(Wv[:, cols]),
        }
        if has_bias:
            m["bq"] = np.ascontiguousarray(bq[cols])
            m["bk"] = np.ascontiguousarray(bk[cols])
            m["bv"] = np.ascontiguousarray(bv[cols])
        if has_mask:
            m["m8t"] = np.ascontiguousarray(mask[b, 0].T * 8.0)
        in_maps.append(m)
    return in_maps, has_bias, has_mask


def make_in_maps(inputs):
    return shard_inputs(**{k: inputs[k] for k in
                           ("query", "key", "value", "mask", "Wq", "bq",
                            "Wk", "bk", "Wv", "bv", "batch_size", "num_heads")})[0]


def assemble(results):
    full = np.empty((B * SEQ, D), dtype=np.float32)
    for c in range(8):
        b, g = divmod(c, 2)
        full[b * SEQ:(b + 1) * SEQ, g * DG:(g + 1) * DG] = results[c]["out"]
    return full


def kernel(query, key, value, mask, Wq, bq, Wk, bk, Wv, bv,
           batch_size=B, num_heads=N_HEADS, _trace=False, _trace_kwargs=None):
    in_maps, has_bias, has_mask = shard_inputs(
        query, key, value, mask, Wq, bq, Wk, bk, Wv, bv, batch_size, num_heads)
    nc = _get_nc(has_bias, has_mask)
    res = run_bass_kernel_spmd(nc, in_maps, list(range(8)), trace=_trace,
                               **(_trace_kwargs or {}))
    full = assemble(res.results)
    if _trace:
        return full, res
    return full
